# revision 26
# baseline (speedup 1.0000x reference)
"""Trainium2 Bass kernel for 16-head self-attention (B=4, L=2048, D=1024).

Sharding: 8 cores = 4 batches x 2 head-groups (8 heads each). Each core
computes qkv projection, attention and a partial out-projection for its
(batch, head-group); the host sums the two head-group partials per batch.

Per-core pipeline (bf16 matmuls, fp32 PSUM accumulation):
  prologue: v in [L, head] layout (ACT evacuates), then qT/kT for all 4
            head pairs (ACT does the bias-adds) -- PE-dense, ACT mostly
            idle, so softmax work can saturate ACT later.
  attention per (pair, i-block): software-pipelined j-loop
            scores (row-group concurrent MM pair) -> exp -> AV with
            ones-column denominator accumulation. The exp is split
            between ACT (exact, scale fused) and DVE (Schraudolph
            bit-trick emitted directly as bf16 bits via an int16
            affine+convert: ~0.9% rms multiplicative noise, which
            averages out through softmax) so neither engine paces the
            PE. Normalize: denominator rows copied to SBUF, reciprocal,
            DRAM-bounce broadcast; the whole chain is deferred into the
            next block's j-loop so it never head-of-line-blocks DVE.
  out-proj: y = ot.T @ W_out partial per i-chunk, ACT evacuates PSUM,
            streamed to DRAM.
"""
import sys

sys.path.insert(0, "/opt/trn_rl_repo")

import numpy as np
import ml_dtypes

import concourse.bacc as bacc
import concourse.mybir as mybir
import concourse.tile as tile
from concourse.bass_utils import run_bass_kernel_spmd

F32 = mybir.dt.float32
BF16 = mybir.dt.bfloat16
I16 = mybir.dt.int16
EXP = mybir.ActivationFunctionType.Exp
COPY = mybir.ActivationFunctionType.Copy
IDENT = mybir.ActivationFunctionType.Identity
MULT = mybir.AluOpType.mult
ADD = mybir.AluOpType.add

B = 4
L = 2048
D = 1024
HEADS = 16
DH = 64
SCALE = DH ** -0.5
N_CORES = 8
HC = HEADS // 2          # heads per core = 8
NPAIR = HC // 2          # head pairs per core = 4
ND = D // 128            # 8 d-chunks
NL = L // 128            # 16 l-chunks
DV = HC * 65             # v_aug width = 520

# Schraudolph exp2 affine emitted as bf16 bits: bits_i16 = rint(x*A + B)
LOG2E = float(np.log2(np.e))
SCH_A = SCALE * LOG2E * 128.0
SCH_B = 127.0 * 128.0 - 366393.0 / 65536.0
# j-chunks whose exp runs on DVE (bit-trick); rest on ACT (exact)
DVE_JS = frozenset((2, 4, 7, 9, 12, 14))

_CACHE = {}


def _build():
    nc = bacc.Bacc("TRN2", target_bir_lowering=False)

    xT_d = nc.dram_tensor("xT", [D, L], BF16, kind="ExternalInput")
    wqk_d = nc.dram_tensor("wqk", [8, 128, ND, 128], BF16,
                           kind="ExternalInput")
    wv_d = nc.dram_tensor("wv", [D, 512], BF16, kind="ExternalInput")
    wout_d = nc.dram_tensor("wout", [512, 1024], BF16, kind="ExternalInput")
    bqk_d = nc.dram_tensor("bqk", [128, 8], F32, kind="ExternalInput")
    y_d = nc.dram_tensor("y", [L, D], F32, kind="ExternalOutput")

    with tile.TileContext(nc) as tc:
        with (
            tc.tile_pool(name="persist", bufs=1) as pp,
            tc.tile_pool(name="dstage", bufs=8, space="DRAM") as dpool,
        ):
            qt = [pp.tile([128, L], BF16, tag=f"qt{c}", name=f"qt{c}")
                  for c in range(NPAIR)]
            kt = [pp.tile([128, L], BF16, tag=f"kt{c}", name=f"kt{c}")
                  for c in range(NPAIR)]
            vt = [pp.tile([128, DV], BF16, tag=f"v{l}", name=f"v{l}")
                  for l in range(NL)]
            ot = [pp.tile([128, L], BF16, tag=f"ot{c}", name=f"ot{c}")
                  for c in range(NPAIR)]
            wout = [pp.tile([128, 1024], BF16, tag=f"wo{c}", name=f"wo{c}")
                    for c in range(NPAIR)]
            bias_all = pp.tile([128, 8], F32, tag="bias", name="bias")

            # ---------------- prologue: qkv projection ----------------
            with (
                tc.tile_pool(name="ph1", bufs=1) as p1,
                tc.tile_pool(name="wstream", bufs=8) as wsp,
                tc.tile_pool(name="acc1", bufs=8, space="PSUM") as acc1,
            ):
                xt = [p1.tile([128, L], BF16, tag=f"xt{d}", name=f"xt{d}")
                      for d in range(ND)]
                wv = [p1.tile([128, 512], BF16, tag=f"wv{d}", name=f"wv{d}")
                      for d in range(ND)]
                # xt[0] lands in column-quarters so the d-major v fill
                # can start on its first 128-col slice ~1.5us earlier.
                for q in range(4):
                    nc.sync.dma_start(
                        xt[0][:, q * 512:(q + 1) * 512],
                        xT_d[0:128, q * 512:(q + 1) * 512])
                nc.sync.dma_start(wv[0][:], wv_d[0:128, :])
                for d in range(1, ND):
                    nc.sync.dma_start(
                        xt[d][:], xT_d[d * 128:(d + 1) * 128, :])
                    nc.sync.dma_start(
                        wv[d][:], wv_d[d * 128:(d + 1) * 128, :])
                nc.sync.dma_start(bias_all[:], bqk_d[:, :])

                # v natural layout with per-head ones column (65-stride).
                # First 6 l-chunks run d-major so the PE streams behind the
                # incoming xT DMA instead of stalling on the full 4MB.
                def v_evac(l, ps):
                    v3 = vt[l][:].rearrange("p (h w) -> p h w", w=65)
                    nc.scalar.activation(
                        v3[:, :, 0:64],
                        ps[:].rearrange("p (h w) -> p h w", w=64), COPY)
                    nc.vector.memset(v3[:, :, 64:65], 1.0)

                NV0 = 8
                ps0 = [acc1.tile([128, 512], F32, tag="acc", name="acc")
                       for _ in range(NV0)]
                for d in range(ND):
                    for l in range(NV0):
                        nc.tensor.matmul(
                            ps0[l][:], xt[d][:, l * 128:(l + 1) * 128],
                            wv[d][:], start=(d == 0), stop=(d == ND - 1))
                for l in range(NV0):
                    v_evac(l, ps0[l])
                for l in range(NV0, NL):
                    ps = acc1.tile([128, 512], F32, tag="acc", name="acc")
                    for d in range(ND):
                        nc.tensor.matmul(
                            ps[:], xt[d][:, l * 128:(l + 1) * 128], wv[d][:],
                            start=(d == 0), stop=(d == ND - 1))
                    v_evac(l, ps)

                # qT (t=0..3) and kT (t=4..7) chunks: [128, L] each,
                # ordered pairwise so attention(c) unblocks early.
                for c in range(NPAIR):
                    nc.sync.dma_start(
                        wout[c][:], wout_d[c * 128:(c + 1) * 128, :])
                for t in (0, 4, 1, 5, 2, 6, 3, 7):
                    dst = qt[t] if t < 4 else kt[t - 4]
                    w = wsp.tile([128, ND, 128], BF16, tag="wqk", name="wqk")
                    nc.sync.dma_start(w[:], wqk_d[t])
                    psums = [acc1.tile([128, 512], F32, tag="acc", name="acc")
                             for _ in range(4)]
                    for d in range(ND):
                        for n in range(4):
                            nc.tensor.matmul(
                                psums[n][:], w[:, d, :],
                                xt[d][:, n * 512:(n + 1) * 512],
                                start=(d == 0), stop=(d == ND - 1))
                    for n in range(4):
                        nc.scalar.activation(
                            dst[:, n * 512:(n + 1) * 512], psums[n][:],
                            IDENT, bias=bias_all[:, t:t + 1])

            # ---------------- attention ----------------
            if True:
                with (
                    tc.tile_pool(name="ppool", bufs=6) as ppl,
                    tc.tile_pool(name="npool", bufs=8) as npl,
                    tc.tile_pool(name="ps_s", bufs=2, space="PSUM") as pss,
                    tc.tile_pool(name="ps_o", bufs=4, space="PSUM") as pso,
                ):
                    # Two-stage deferred normalize: stage 1 (denominator
                    # copies + reciprocals + DMA bounce) and stage 2
                    # (broadcast muls into ot) of block k both run inside
                    # block k+1's j-loop, so they never head-of-line-block
                    # the DVE exp stream and the DMA round-trip is hidden.
                    front_q = []
                    mul_q = []

                    def flush(q):
                        while q:
                            q.pop(0)()

                    # Flat software-pipelined loop over all (pair,
                    # i-block, j) steps: S/E lead, AV lags 3 steps and
                    # crosses block boundaries, so the PE stream is fully
                    # periodic with no per-block refill bubble.
                    steps = [(c, ib, j)
                             for c in range(NPAIR)
                             for ib in range(4)
                             for j in range(NL)]
                    blocks = {}

                    def get_block(c, ib):
                        if (c, ib) not in blocks:
                            blocks[(c, ib)] = (
                                pso.tile([128, 512], F32, tag="o", name="o"),
                                pso.tile([128, 512], F32, tag="o", name="o"))
                        return blocks[(c, ib)]

                    def scores(c, ib, j):
                        js = slice(j * 128, (j + 1) * 128)
                        isl = slice(ib * 512, ib * 512 + 512)
                        s = pss.tile([128, 1024], F32, tag="s", name="s")
                        nc.tensor.matmul(
                            s[:, 0:512], kt[c][0:64, js],
                            qt[c][0:64, isl], start=True, stop=True)
                        nc.tensor.matmul(
                            s[:, 512:1024], kt[c][64:128, js],
                            qt[c][64:128, isl], start=True, stop=True)
                        return s

                    def expj(s, j):
                        p = ppl.tile([128, 1024], BF16, tag="p", name="p")
                        if j in DVE_JS:
                            nc.vector.tensor_scalar(
                                p[:].bitcast(I16), s[:],
                                SCH_A, SCH_B, MULT, ADD)
                        else:
                            nc.scalar.activation(
                                p[:], s[:], EXP, scale=float(SCALE))
                        return p

                    def av(c, ib, j, p):
                        o_h, o_g = get_block(c, ib)
                        st, sp = j == 0, j == NL - 1
                        va = vt[j][:, c * 130:c * 130 + 65]
                        vb = vt[j][:, c * 130 + 65:c * 130 + 130]
                        nc.tensor.matmul(o_h[0:65, :], va, p[:, 0:512],
                                         start=st, stop=sp)
                        nc.tensor.matmul(o_g[0:65, :], vb, p[:, 512:1024],
                                         start=st, stop=sp)
                        if sp:
                            finish_block(c, ib, o_h, o_g)

                    def finish_block(c, ib, o_h, o_g):
                        isl = slice(ib * 512, ib * 512 + 512)
                        rbs = [npl.tile([64, 512], F32, tag="rb", name="rb")
                               for _ in range(2)]

                        def mk_front(o_h=o_h, o_g=o_g, rbs=rbs):
                            for o_t, rb in zip((o_h, o_g), rbs):
                                dcp = npl.tile([1, 512], F32, tag="dcp",
                                               name="dcp")
                                rcp = npl.tile([1, 512], F32, tag="rcp",
                                               name="rcp")
                                nc.vector.tensor_copy(dcp[:], o_t[64:65, :])
                                nc.vector.reciprocal_approx_fast(
                                    out=rcp[:], in_=dcp[:])
                                dst = dpool.tile([1, 512], F32, tag="rst",
                                                 name="rst")
                                nc.sync.dma_start(dst[:], rcp[:])
                                nc.sync.dma_start(
                                    rb[:], dst[:].to_broadcast([64, 512]))

                        def mk_norm(c=c, isl=isl, o_h=o_h, o_g=o_g, rbs=rbs):
                            nc.vector.tensor_mul(
                                ot[c][0:64, isl], o_h[0:64, :], rbs[0][:])
                            nc.vector.tensor_mul(
                                ot[c][64:128, isl], o_g[0:64, :], rbs[1][:])
                        front_q.append(mk_front)
                        mul_q.append(mk_norm)

                    pend = []
                    for k, (c, ib, j) in enumerate(steps):
                        s = scores(c, ib, j)
                        p = expj(s, j)
                        pend.append((c, ib, j, p))
                        if k >= 3:
                            av(*pend[k - 3])
                        if j == 5:
                            flush(front_q)
                        elif j == 12:
                            flush(mul_q)
                    for k in range(len(steps) - 3, len(steps)):
                        av(*pend[k])
                    flush(front_q)
                    flush(mul_q)

                    # ---------------- out projection ----------------
                    # psm reuses the pss pool buffers (same shape/banks), so
                    # no PSUM scope transition (no engine drains, no HAM
                    # cooldown) between attention and the projection.
                    for i in range(NL):
                        psm = pss.tile([128, 1024], F32, tag="s", name="s")
                        for cc in range(NPAIR):
                            for m in range(2):
                                nc.tensor.matmul(
                                    psm[:, m * 512:(m + 1) * 512],
                                    ot[cc][:, i * 128:(i + 1) * 128],
                                    wout[cc][:, m * 512:(m + 1) * 512],
                                    start=(cc == 0), stop=(cc == NPAIR - 1))
                        yst = npl.tile([128, 1024], F32, tag="yst",
                                       name="yst")
                        nc.scalar.activation(yst[:], psm[:], COPY)
                        nh = 4 if i == NL - 1 else 2
                        for h in range(nh):
                            w = 1024 // nh
                            nc.sync.dma_start(
                                y_d[i * 128:(i + 1) * 128,
                                    h * w:(h + 1) * w],
                                yst[:, h * w:(h + 1) * w])

    nc.finalize()
    return nc


def _get_nc():
    if "nc" not in _CACHE:
        _CACHE["nc"] = _build()
    return _CACHE["nc"]


def _make_in_maps(x, W_qkv, b_qkv, W_out):
    xT = [np.ascontiguousarray(x[b].T).astype(ml_dtypes.bfloat16)
          for b in range(B)]
    in_maps = []
    for b in range(B):
        for g in range(2):
            sl = slice(g * 512, (g + 1) * 512)
            wqk_c = np.concatenate(
                [W_qkv[:, sl],
                 W_qkv[:, 1024 + g * 512:1024 + (g + 1) * 512]],
                axis=1).astype(ml_dtypes.bfloat16)
            # [1024, 1024] -> [t, p, d, c] with src index [d*128+p, t*128+c]
            wqk_r = np.ascontiguousarray(
                wqk_c.reshape(8, 128, 8, 128).transpose(2, 1, 0, 3))
            wv_c = np.ascontiguousarray(
                W_qkv[:, 2048 + g * 512:2048 + (g + 1) * 512]).astype(
                    ml_dtypes.bfloat16)
            wout_c = np.ascontiguousarray(W_out[sl, :]).astype(
                ml_dtypes.bfloat16)
            bqk_c = np.ascontiguousarray(
                np.concatenate(
                    [b_qkv[g * 512:(g + 1) * 512],
                     b_qkv[1024 + g * 512:1024 + (g + 1) * 512]])
                .reshape(8, 128).T.astype(np.float32))
            in_maps.append({
                "xT": xT[b],
                "wqk": wqk_r,
                "wv": wv_c,
                "wout": wout_c,
                "bqk": bqk_c,
            })
    return in_maps


def kernel(x, W_qkv, b_qkv, W_out, b_out):
    x = np.asarray(x, dtype=np.float32)
    W_qkv = np.asarray(W_qkv, dtype=np.float32)
    b_qkv = np.asarray(b_qkv, dtype=np.float32)
    W_out = np.asarray(W_out, dtype=np.float32)
    b_out = np.asarray(b_out, dtype=np.float32)

    nc = _get_nc()
    in_maps = _make_in_maps(x, W_qkv, b_qkv, W_out)
    res = run_bass_kernel_spmd(nc, in_maps, core_ids=list(range(N_CORES)))

    # v-bias flows additively through softmax (rows sum to 1): + b_v @ W_out
    y_bias = b_qkv[2048:3072] @ W_out + b_out
    out = np.empty((B, L, D), dtype=np.float32)
    for b in range(B):
        out[b] = res.results[2 * b]["y"] + res.results[2 * b + 1]["y"] + y_bias
    return out
